# revision 6
# baseline (speedup 1.0000x reference)
"""Trainium2 Bass kernel for NT-Xent contrastive loss (N=4096, D=256).

loss = mean_i(log(sum_{k!=i} exp(sim(r_i,r_k)/T)) - sim(r_i, r_{i+N mod 2N})/T)
with r = row-l2-normalized concat(emb_i, emb_j), T = 0.5.

Sharding: rows of the [8192, 8192] similarity matrix are split across the
8 cores (1024 rows each, passed per-core as `my_rows`). Every core builds
the full normalized transposed reps [256, 8192] (bf16) in SBUF, computes
its row-block of the Gram matrix on the PE in [128, 512] psum tiles,
does exp+row-sum on the Scalar engine (fused accum), excludes the diagonal
analytically (exp(2*||rho_r||^2)), takes log, and reduces. The positive
term is computed directly from normalized row pairs on the Vector engine
(identical on every core; each core subtracts 1/8 of it). Host sums the
8 [128, 2] partials.
"""

import os
import numpy as np

import concourse.bass as bass
import concourse.bacc as bacc
import concourse.tile as tile
from concourse import mybir
from concourse.bass_utils import run_bass_kernel_spmd
from concourse.masks import make_identity
from contextlib import ExitStack

N = 4096
D = 256
TWO_N = 2 * N
N_CORES = 8
ROWS_PER_CORE = TWO_N // N_CORES  # 1024
M_TILES = ROWS_PER_CORE // 128    # 8
FULL_TILES = TWO_N // 128         # 64 (32 from emb_i, 32 from emb_j)
KC = 2                            # 256 = 2 chunks of 128 on partitions

F32 = mybir.dt.float32
BF16 = mybir.dt.bfloat16
ALU = mybir.AluOpType
ACT = mybir.ActivationFunctionType
AXX = mybir.AxisListType


def _emit(nc, tc, ctx, emb_i, emb_j, my_rows, out):
    persist = ctx.enter_context(tc.tile_pool(name="persist", bufs=1))
    work = ctx.enter_context(tc.tile_pool(name="work", bufs=3))
    small = ctx.enter_context(tc.tile_pool(name="small", bufs=2))
    psum_tr = ctx.enter_context(tc.tile_pool(name="psum_tr", bufs=2, space="PSUM"))
    psum_mm = ctx.enter_context(tc.tile_pool(name="psum_mm", bufs=3, space="PSUM"))

    # ---- persistent SBUF ----
    # repsT[k_part, kc, tile, col]: normalized reps, transposed, bf16.
    repsT = persist.tile([128, KC, FULL_TILES, 128], BF16)
    lhsT = persist.tile([128, KC, M_TILES, 128], BF16)
    ident = persist.tile([128, 128], BF16)
    make_identity(nc, ident)

    # p-major staging: raw_full[:, t, :]: t in 0..31 -> emb_i row 32p+t,
    # t in 32..63 -> emb_j row 32p+(t-32). raw_my[:, m, :] -> my row 8p+m.
    raw_full = persist.tile([128, FULL_TILES, D], BF16)
    raw_my = persist.tile([128, M_TILES, D], BF16)
    rn_full = persist.tile([128, FULL_TILES, D], BF16)
    rn_my = persist.tile([128, M_TILES, D], BF16)

    pos_stage = persist.tile([128, 32], F32)
    diag_stage = persist.tile([128, M_TILES], F32)
    ld_stage = persist.tile([128, M_TILES], F32)
    fin = persist.tile([128, 2], F32)

    # ---- loads (SWDGE casts f32 -> bf16 in flight; p-major = one big
    # contiguous chunk per partition per DMA) ----
    ei = emb_i.ap().rearrange("(p t) d -> p t d", p=128)  # [128, 32, 256]
    ej = emb_j.ap().rearrange("(p t) d -> p t d", p=128)
    mr = my_rows.ap().rearrange("(p t) d -> p t d", p=128)  # [128, 8, 256]
    nc.gpsimd.dma_start(out=raw_my[:, :, :], in_=mr)
    for h in range(2):
        nc.gpsimd.dma_start(
            out=raw_full[:, 16 * h:16 * (h + 1), :], in_=ei[:, 16 * h:16 * (h + 1), :])
    for h in range(2):
        nc.gpsimd.dma_start(
            out=raw_full[:, 32 + 16 * h:32 + 16 * (h + 1), :],
            in_=ej[:, 16 * h:16 * (h + 1), :])

    def norm_group(raw, rn, tiles, dstT, dst_tiles):
        """Normalize rows of `raw[:, t, :]` for t in tiles into rn, and
        transpose into dstT[:, kc, dst_tile, :]."""
        g = len(tiles)
        ss = small.tile([128, g], F32, tag="ss")
        for j, t in enumerate(tiles):
            junk = work.tile([128, D], BF16, tag="sqjunk")
            nc.vector.scalar_tensor_tensor(
                out=junk[:, :], in0=raw[:, t, :], scalar=1.0, in1=raw[:, t, :],
                op0=ALU.bypass, op1=ALU.mult, accum_out=ss[:, j:j + 1])
        lnss = small.tile([128, g], F32, tag="lnss")
        nc.scalar.activation(out=lnss[:, :], in_=ss[:, :], func=ACT.Ln)
        inv = small.tile([128, g], F32, tag="inv")
        nc.scalar.activation(out=inv[:, :], in_=lnss[:, :], func=ACT.Exp, scale=-0.5)
        for j, t in enumerate(tiles):
            nc.vector.tensor_scalar(
                out=rn[:, t, :], in0=raw[:, t, :], scalar1=inv[:, j:j + 1],
                scalar2=None, op0=ALU.mult)
            ps = psum_tr.tile([128, KC, 128], BF16, tag="ptr")
            for kc in range(KC):
                nc.tensor.transpose(
                    out=ps[:, kc, :], in_=rn[:, t, kc * 128:(kc + 1) * 128],
                    identity=ident[:, :])
            dt_ = dst_tiles[j]
            nc.vector.tensor_copy(dstT[:, :, dt_, :], ps[:, :, :])

    # my rows first (unblocks matmuls earliest)
    norm_group(raw_my, rn_my, list(range(M_TILES)), lhsT, list(range(M_TILES)))
    # diag logits for my rows: 2*||rho_r||^2
    for m in range(M_TILES):
        junk = work.tile([128, D], BF16, tag="sqjunk")
        nc.vector.scalar_tensor_tensor(
            out=junk[:, :], in0=rn_my[:, m, :], scalar=2.0, in1=rn_my[:, m, :],
            op0=ALU.mult, op1=ALU.mult, accum_out=diag_stage[:, m:m + 1])
    ediag = persist.tile([128, M_TILES], F32)
    nc.scalar.activation(out=ediag[:, :], in_=diag_stage[:, :], func=ACT.Exp)

    # full reps, in groups of 8 tiles pairing (t, t+32) for the positive term
    for gi in range(8):
        tiles = [4 * gi + j for j in range(4)] + [32 + 4 * gi + j for j in range(4)]
        norm_group(raw_full, rn_full, tiles, repsT, tiles)
        for j in range(4):
            t = 4 * gi + j
            junk = work.tile([128, D], BF16, tag="sqjunk")
            # sum over both halves of the positive logits for pair r:
            # pos contribution = 4 * dot(rho_i_r, rho_j_r)
            nc.vector.scalar_tensor_tensor(
                out=junk[:, :], in0=rn_full[:, t, :], scalar=4.0,
                in1=rn_full[:, t + 32, :],
                op0=ALU.mult, op1=ALU.mult, accum_out=pos_stage[:, t:t + 1])

    # ---- main: G row-block, exp+rowsum ----
    for m in range(M_TILES):
        den = small.tile([128, 8], F32, tag="den")
        for nci in range(8):
            ps = psum_mm.tile([128, 1024], F32, tag="mm")
            for half in range(2):
                tb = nci * 8 + half * 4
                for kc in range(KC):
                    nc.tensor.matmul(
                        out=ps[:, half * 512:(half + 1) * 512],
                        lhsT=lhsT[:, kc, m, :],
                        rhs=repsT[:, kc, tb:tb + 4, :],
                        start=(kc == 0), stop=(kc == 1))
            ej_ = work.tile([128, 1024], F32, tag="expjunk")
            nc.scalar.activation(
                out=ej_[:, :], in_=ps[:, :], func=ACT.Exp, scale=2.0,
                accum_out=den[:, nci:nci + 1])
        densum = small.tile([128, 1], F32, tag="densum")
        nc.vector.tensor_reduce(out=densum[:, :], in_=den[:, :], axis=AXX.X, op=ALU.add)
        dex = small.tile([128, 1], F32, tag="dex")
        nc.vector.tensor_sub(dex[:, :], densum[:, :], ediag[:, m:m + 1])
        nc.scalar.activation(out=ld_stage[:, m:m + 1], in_=dex[:, :], func=ACT.Ln)

    nc.vector.tensor_reduce(out=fin[:, 0:1], in_=ld_stage[:, :], axis=AXX.X, op=ALU.add)
    nc.vector.tensor_reduce(out=fin[:, 1:2], in_=pos_stage[:, :], axis=AXX.X, op=ALU.add)
    nc.sync.dma_start(out=out.ap(), in_=fin[:, :])


_CACHED = None


def _build():
    global _CACHED
    if _CACHED is not None:
        return _CACHED
    nc = bacc.Bacc("TRN2", target_bir_lowering=False, debug=False,
                   enable_asserts=False, num_devices=N_CORES)
    emb_i = nc.dram_tensor("emb_i", [N, D], F32, kind="ExternalInput")
    emb_j = nc.dram_tensor("emb_j", [N, D], F32, kind="ExternalInput")
    my_rows = nc.dram_tensor("my_rows", [ROWS_PER_CORE, D], F32, kind="ExternalInput")
    out = nc.dram_tensor("out", [128, 2], F32, kind="ExternalOutput")
    with tile.TileContext(nc) as tc:
        with ExitStack() as ctx:
            _emit(nc, tc, ctx, emb_i, emb_j, my_rows, out)
    nc.compile()
    _CACHED = nc
    return nc


LAST_EXEC_NS = None
LAST_TRACE = None


def kernel(emb_i, emb_j, batch_size):
    global LAST_EXEC_NS, LAST_TRACE
    emb_i = np.ascontiguousarray(np.asarray(emb_i), dtype=np.float32)
    emb_j = np.ascontiguousarray(np.asarray(emb_j), dtype=np.float32)
    assert emb_i.shape == (N, D) and emb_j.shape == (N, D)
    concat = np.concatenate([emb_i, emb_j], axis=0)

    nc = _build()
    in_maps = []
    for c in range(N_CORES):
        in_maps.append({
            "emb_i": emb_i,
            "emb_j": emb_j,
            "my_rows": np.ascontiguousarray(
                concat[c * ROWS_PER_CORE:(c + 1) * ROWS_PER_CORE]),
        })
    trace = bool(int(os.environ.get("KERNEL_TRACE", "0")))
    res = run_bass_kernel_spmd(nc, in_maps, list(range(N_CORES)), trace=trace)
    LAST_EXEC_NS = res.exec_time_ns
    if res.instructions_and_trace is not None:
        LAST_TRACE = res.instructions_and_trace[1]

    total = 0.0
    for c in range(N_CORES):
        o = np.asarray(res.results[c]["out"], dtype=np.float64)
        total += o[:, 0].sum() - 0.125 * o[:, 1].sum()
    return np.array(total / TWO_N, dtype=np.float32)
